# revision 1
# baseline (speedup 1.0000x reference)
"""GCN layer kernel for Trainium2 (8 NeuronCores).

Strategy (per sharding_hint): shard node rows across the 8 cores for the
dense projection Z = X @ W (the FLOP-heavy part) on the TensorEngines.
W [512,512] is replicated. To avoid on-device transposes, the host passes
X^T and the device computes OutT = W^T @ X^T; the host transposes back.
The irregular COO scatter-add (segment_sum over 800k random edges) is done
host-side as a CSR SpMM, followed by ReLU.
"""

import numpy as np

N_NODES = 50000
M_IN = 512
H_OUT = 512
N_CORES = 8
# per-core padded column count: 13 blocks of 512
COLS = 6656
PAD_NODES = COLS * N_CORES  # 53248

_compiled = {}


def _build_nc():
    from concourse import bacc, mybir
    from concourse import tile

    f32 = mybir.dt.float32
    bf16 = mybir.dt.bfloat16
    nc = bacc.Bacc(None, debug=False)

    xt = nc.declare_dram_parameter("xt", [M_IN, COLS], bf16, isOutput=False)
    w = nc.declare_dram_parameter("w", [M_IN, H_OUT], bf16, isOutput=False)
    outt = nc.declare_dram_parameter("out", [H_OUT, COLS], f32, isOutput=True)

    KC = M_IN // 128  # 4 contraction chunks
    NC_ = H_OUT // 128  # 4 output-row chunks
    NB = COLS // 512  # 13 column blocks

    with tile.TileContext(nc) as tc:
        with (
            tc.tile_pool(name="wpool", bufs=1) as wpool,
            tc.tile_pool(name="xpool", bufs=3) as xpool,
            tc.tile_pool(name="opool", bufs=4) as opool,
            tc.tile_pool(name="psum", bufs=4, space="PSUM") as pp,
        ):
            wt = wpool.tile([128, KC, H_OUT], bf16)
            for k in range(KC):
                nc.sync.dma_start(wt[:, k, :], w[k * 128 : (k + 1) * 128, :])

            for cb in range(NB):
                xtile = xpool.tile([128, KC, 512], bf16)
                for k in range(KC):
                    nc.sync.dma_start(
                        xtile[:, k, :],
                        xt[k * 128 : (k + 1) * 128, cb * 512 : (cb + 1) * 512],
                    )
                for n in range(NC_):
                    acc = pp.tile([128, 512], f32)
                    for k in range(KC):
                        nc.tensor.matmul(
                            acc[:],
                            wt[:, k, n * 128 : (n + 1) * 128],
                            xtile[:, k, :],
                            start=(k == 0),
                            stop=(k == KC - 1),
                        )
                    otile = opool.tile([128, 512], f32)
                    nc.vector.tensor_copy(otile[:], acc[:])
                    nc.sync.dma_start(
                        outt[n * 128 : (n + 1) * 128, cb * 512 : (cb + 1) * 512],
                        otile[:],
                    )
    nc.compile()
    return nc


def _get_nc():
    if "nc" not in _compiled:
        _compiled["nc"] = _build_nc()
    return _compiled["nc"]


def kernel(X, W, edge_src, edge_dst, edge_vals):
    import scipy.sparse as sp
    from concourse.bass_utils import run_bass_kernel_spmd

    X = np.asarray(X, dtype=np.float32)
    W = np.ascontiguousarray(np.asarray(W, dtype=np.float32))
    edge_src = np.asarray(edge_src)
    edge_dst = np.asarray(edge_dst)
    edge_vals = np.asarray(edge_vals, dtype=np.float32)

    import ml_dtypes

    # host pre-transpose + pad so the device needs no transposes; bf16 halves
    # the upload and uses the fast TensorEngine path (rel err ~2e-3)
    bf = ml_dtypes.bfloat16
    XT = np.zeros((M_IN, PAD_NODES), dtype=bf)
    XT[:, :N_NODES] = X.T.astype(bf)
    W = W.astype(bf)

    in_maps = [
        {"xt": np.ascontiguousarray(XT[:, i * COLS : (i + 1) * COLS]), "w": W}
        for i in range(N_CORES)
    ]

    nc = _get_nc()
    res = run_bass_kernel_spmd(nc, in_maps, core_ids=list(range(N_CORES)))
    outs = res.results
    ZT = np.concatenate([np.asarray(outs[i]["out"]) for i in range(N_CORES)], axis=1)
    Z = np.ascontiguousarray(ZT[:, :N_NODES].T)  # [N, H]

    A = sp.csr_matrix(
        (edge_vals, (edge_dst.astype(np.int64), edge_src.astype(np.int64))),
        shape=(N_NODES, N_NODES),
    )
    agg = A @ Z
    return np.maximum(agg, 0.0).astype(np.float32)



# revision 5
# speedup vs baseline: 117.8790x; 117.8790x over previous
"""GCN layer (A_hat @ (X W) with ReLU) fully on-device for Trainium2, 8 cores.

Math: out = relu(segment_sum(Z[edge_src] * edge_vals, edge_dst)) with Z = X@W.
We use the equivalent order out = relu((A @ X) @ W) so the gather table is the
input X (replicated to every core) and no cross-core collective is needed.

Sharding: destination nodes are sharded across the 8 cores (graph
partitioning per the hint). Each core owns 49 blocks of 128 dst nodes.
Per block, the device:
  1. bulk-gathers the source rows of the block's edges from X (bf16) via
     two dma_gather calls (X is split in two tables so row indices fit the
     gather engine's int16 index format),
  2. builds a [128 edges x 128 dst] indicator matrix per 128-edge chunk on
     the VectorEngine (indicator[e,d] = edge_val[e] if dst_local[e]==d),
  3. accumulates agg = indicator^T @ msg on the TensorEngine into PSUM
     (the segment-sum becomes a chain of matmuls, race-free),
  4. transposes agg (TensorE) and multiplies by W, applies ReLU (ScalarE),
     writes the finished [128, 512] output rows once. No scatter anywhere.

Host work is only layout: degree-balanced dst->block permutation, edge
sorting/padding, int16 index packing, and the inverse row permutation on
the way out.
"""

import numpy as np

N = 50000
F = 512
E = 800000
CORES = 8
SPLIT = 32768          # X row split so gather indices fit int16
NLO = SPLIT
NHI = N - SPLIT        # 17232
BPC = 49               # dst blocks per core
BLOCKS = CORES * BPC   # 392
PADN = BLOCKS * 128    # 50176
T_LO, T_HI = 11, 7     # static 128-edge chunks per block per table
T = T_LO + T_HI
CAP_LO, CAP_HI = T_LO * 128, T_HI * 128
SLOTS = T * 128
ICOL_LO, ICOL_HI = CAP_LO // 16, CAP_HI // 16
ICOLS = ICOL_LO + ICOL_HI
NCHUNK = BPC * T

_compiled = {}


def _build_nc():
    from concourse import bacc, mybir
    from concourse import tile

    f32 = mybir.dt.float32
    bf16 = mybir.dt.bfloat16
    i16 = mybir.dt.int16
    AF = mybir.ActivationFunctionType
    OP = mybir.AluOpType

    nc = bacc.Bacc(None, debug=False)

    xlo_d = nc.declare_dram_parameter("xlo", [NLO, F], bf16, isOutput=False)
    xhi_d = nc.declare_dram_parameter("xhi", [NHI, F], bf16, isOutput=False)
    idx_d = nc.declare_dram_parameter("idx", [128, BPC * ICOLS], i16, isOutput=False)
    dloc_d = nc.declare_dram_parameter("dloc", [128, NCHUNK], f32, isOutput=False)
    vloc_d = nc.declare_dram_parameter("vloc", [128, NCHUNK], f32, isOutput=False)
    wt_d = nc.declare_dram_parameter("wt", [128, 4, F], bf16, isOutput=False)
    iota_d = nc.declare_dram_parameter("iota", [128, 128], bf16, isOutput=False)
    ident_d = nc.declare_dram_parameter("ident", [128, 128], bf16, isOutput=False)
    out_d = nc.declare_dram_parameter("out", [BPC * 128, F], f32, isOutput=True)

    with tile.TileContext(nc) as tc:
        with (
            tc.tile_pool(name="const", bufs=1) as cp,
            tc.tile_pool(name="msgp", bufs=3) as mp,
            tc.tile_pool(name="indp", bufs=3) as ip,
            tc.tile_pool(name="smallp", bufs=3) as sp,
            tc.tile_pool(name="accp", bufs=2, space="PSUM") as pa,
            tc.tile_pool(name="tpp", bufs=2, space="PSUM") as pt,
            tc.tile_pool(name="o2p", bufs=2, space="PSUM") as po,
        ):
            idx_t = cp.tile([128, BPC * ICOLS], i16)
            nc.sync.dma_start(idx_t[:], idx_d[:])
            dloc_t = cp.tile([128, NCHUNK], f32)
            nc.sync.dma_start(dloc_t[:], dloc_d[:])
            vloc_t = cp.tile([128, NCHUNK], f32)
            nc.sync.dma_start(vloc_t[:], vloc_d[:])
            wt_t = cp.tile([128, 4, F], bf16)
            nc.sync.dma_start(wt_t[:], wt_d[:])
            iota_t = cp.tile([128, 128], bf16)
            nc.sync.dma_start(iota_t[:], iota_d[:])
            ident_t = cp.tile([128, 128], bf16)
            nc.sync.dma_start(ident_t[:], ident_d[:])

            for b in range(BPC):
                msg = mp.tile([128, T, F], bf16)
                ib = b * ICOLS
                # the gather ucode handles at most 1024 indices (64 descs)
                # per instruction -> split the lo gather 1024 + 384
                nc.gpsimd.dma_gather(
                    msg[:, 0:8, :], xlo_d[:],
                    idx_t[:, ib : ib + 64], 1024, 1024, F,
                )
                nc.gpsimd.dma_gather(
                    msg[:, 8:T_LO, :], xlo_d[:],
                    idx_t[:, ib + 64 : ib + ICOL_LO], CAP_LO - 1024, CAP_LO - 1024, F,
                )
                nc.gpsimd.dma_gather(
                    msg[:, T_LO:T, :], xhi_d[:],
                    idx_t[:, ib + ICOL_LO : ib + ICOLS], CAP_HI, CAP_HI, F,
                )
                ind = ip.tile([128, T, 128], bf16)
                cb = b * T
                for t in range(T):
                    nc.vector.tensor_scalar(
                        ind[:, t, :], iota_t[:],
                        dloc_t[:, cb + t : cb + t + 1],
                        vloc_t[:, cb + t : cb + t + 1],
                        OP.is_equal, OP.mult,
                    )
                acc = pa.tile([128, F], f32)
                for t in range(T):
                    nc.tensor.matmul(
                        acc[:], ind[:, t, :], msg[:, t, :],
                        start=(t == 0), stop=(t == T - 1),
                    )
                ag = sp.tile([128, F], bf16, tag="ag")
                nc.vector.tensor_copy(ag[:], acc[:])
                tp = pt.tile([128, F], bf16)
                for k in range(4):
                    nc.tensor.transpose(
                        tp[:, k * 128 : (k + 1) * 128],
                        ag[:, k * 128 : (k + 1) * 128],
                        ident_t[:],
                    )
                tpsb = sp.tile([128, F], bf16, tag="tpsb")
                nc.vector.tensor_copy(tpsb[:], tp[:])
                o2 = po.tile([128, F], f32)
                for k in range(4):
                    nc.tensor.matmul(
                        o2[:], tpsb[:, k * 128 : (k + 1) * 128], wt_t[:, k, :],
                        start=(k == 0), stop=(k == 3),
                    )
                osb = sp.tile([128, F], f32, tag="osb")
                nc.scalar.activation(osb[:], o2[:], AF.Relu)
                nc.sync.dma_start(out_d[b * 128 : (b + 1) * 128, :], osb[:])
    nc.compile()
    return nc


def _get_nc():
    if "nc" not in _compiled:
        _compiled["nc"] = _build_nc()
    return _compiled["nc"]


def _prep_inputs(X, W, edge_src, edge_dst, edge_vals):
    """Host layout: permutation, edge sort/pad, index packing. Returns
    (in_maps, pos) where pos[node] is its row in the concatenated output."""
    import ml_dtypes

    bf = ml_dtypes.bfloat16

    src = np.asarray(edge_src).astype(np.int64)
    dst = np.asarray(edge_dst).astype(np.int64)
    vals = np.asarray(edge_vals).astype(np.float32)
    X = np.asarray(X, dtype=np.float32)
    W = np.asarray(W, dtype=np.float32)

    islo = src < SPLIT
    deg_lo = np.bincount(dst[islo], minlength=N)
    deg_lo_full = np.concatenate([deg_lo, np.zeros(PADN - N, np.int64)])
    order = np.argsort(-deg_lo_full, kind="stable")
    r = np.arange(PADN)
    row = r // BLOCKS
    col = r % BLOCKS
    blk = np.where(row % 2 == 0, col, BLOCKS - 1 - col)
    block_of = np.empty(PADN, np.int64)
    slot_of = np.empty(PADN, np.int64)
    block_of[order] = blk
    slot_of[order] = row
    pos = block_of * 128 + slot_of

    b_e = block_of[dst]
    g_e = b_e * 2 + (~islo).astype(np.int64)
    eo = np.argsort(g_e, kind="stable")
    g_s = g_e[eo]
    src_s = src[eo]
    dst_s = dst[eo]
    val_s = vals[eo]
    counts = np.bincount(g_e, minlength=BLOCKS * 2)
    n_lo_max = counts[0::2].max()
    n_hi_max = counts[1::2].max()
    assert n_lo_max <= CAP_LO and n_hi_max <= CAP_HI, (
        f"block group overflow: lo {n_lo_max}/{CAP_LO} hi {n_hi_max}/{CAP_HI}"
    )
    starts = np.zeros(BLOCKS * 2, np.int64)
    starts[1:] = np.cumsum(counts)[:-1]
    rank = np.arange(E) - starts[g_s]
    b_s = g_s // 2
    hi_s = g_s % 2
    core_s = b_s // BPC
    bb_s = b_s % BPC

    mslot = hi_s * CAP_LO + rank
    chunk = bb_s * T + mslot // 128
    part = mslot % 128
    icol = bb_s * ICOLS + hi_s * ICOL_LO + rank // 16
    ipart = rank % 16
    idxval = np.where(hi_s == 1, src_s - SPLIT, src_s).astype(np.int16)

    idx_base = np.zeros((CORES, 16, BPC * ICOLS), np.int16)
    idx_base[core_s, ipart, icol] = idxval
    idx = np.ascontiguousarray(np.tile(idx_base, (1, 8, 1)))
    dloc = np.zeros((CORES, 128, NCHUNK), dtype=np.float32)
    vloc = np.zeros((CORES, 128, NCHUNK), dtype=np.float32)
    dloc[core_s, part, chunk] = slot_of[dst_s].astype(np.float32)
    vloc[core_s, part, chunk] = val_s.astype(np.float32)

    xlo = np.ascontiguousarray(X[:SPLIT].astype(bf))
    xhi = np.ascontiguousarray(X[SPLIT:].astype(bf))
    wt = np.ascontiguousarray(
        W.astype(bf).reshape(4, 128, F).transpose(1, 0, 2)
    )
    iota = np.ascontiguousarray(
        np.broadcast_to(np.arange(128), (128, 128)).astype(bf)
    )
    ident = np.eye(128, dtype=bf)

    in_maps = [
        {
            "xlo": xlo,
            "xhi": xhi,
            "idx": np.ascontiguousarray(idx[c]),
            "dloc": np.ascontiguousarray(dloc[c]),
            "vloc": np.ascontiguousarray(vloc[c]),
            "wt": wt,
            "iota": iota,
            "ident": ident,
        }
        for c in range(CORES)
    ]
    return in_maps, pos


def kernel(X, W, edge_src, edge_dst, edge_vals):
    from concourse.bass_utils import run_bass_kernel_spmd

    in_maps, pos = _prep_inputs(X, W, edge_src, edge_dst, edge_vals)
    nc = _get_nc()
    res = run_bass_kernel_spmd(nc, in_maps, core_ids=list(range(CORES)))
    outs = res.results
    full = np.concatenate(
        [np.asarray(outs[c]["out"]) for c in range(CORES)], axis=0
    )  # [PADN, F]
    return np.ascontiguousarray(full[pos[:N]]).astype(np.float32)


# revision 6
# speedup vs baseline: 178.0529x; 1.5105x over previous
"""GCN layer (A_hat @ (X W) with ReLU) fully on-device for Trainium2, 8 cores.

Math: out = relu(segment_sum(Z[edge_src] * edge_vals, edge_dst)) with Z = X@W.
We use the equivalent order out = relu((A @ X) @ W) so the gather table is the
input X (replicated to every core) and no cross-core collective is needed.

Sharding: destination nodes are sharded across the 8 cores (graph
partitioning per the hint). Each core owns 49 blocks of 128 dst nodes.
Per block, the device:
  1. bulk-gathers the source rows of the block's edges from X (bf16) via
     two dma_gather calls (X is split in two tables so row indices fit the
     gather engine's int16 index format),
  2. builds a [128 edges x 128 dst] indicator matrix per 128-edge chunk on
     the VectorEngine (indicator[e,d] = edge_val[e] if dst_local[e]==d),
  3. accumulates agg = indicator^T @ msg on the TensorEngine into PSUM
     (the segment-sum becomes a chain of matmuls, race-free),
  4. transposes agg (TensorE) and multiplies by W, applies ReLU (ScalarE),
     writes the finished [128, 512] output rows once. No scatter anywhere.

Host work is only layout: degree-balanced dst->block permutation, edge
sorting/padding, int16 index packing, and the inverse row permutation on
the way out.
"""

import numpy as np

N = 50000
F = 512
E = 800000
CORES = 8
SPLIT = 32768          # X row split so gather indices fit int16
NLO = SPLIT
NHI = N - SPLIT        # 17232
BPC = 49               # dst blocks per core
BLOCKS = CORES * BPC   # 392
PADN = BLOCKS * 128    # 50176
T_LO, T_HI = 11, 7     # static 128-edge chunks per block per table
T = T_LO + T_HI
CAP_LO, CAP_HI = T_LO * 128, T_HI * 128
SLOTS = T * 128
ICOL_LO, ICOL_HI = CAP_LO // 16, CAP_HI // 16
ICOLS = ICOL_LO + ICOL_HI
NCHUNK = BPC * T

_compiled = {}


def _build_nc():
    from concourse import bacc, mybir
    from concourse import tile

    f32 = mybir.dt.float32
    bf16 = mybir.dt.bfloat16
    i16 = mybir.dt.int16
    AF = mybir.ActivationFunctionType
    OP = mybir.AluOpType

    nc = bacc.Bacc(None, debug=False, num_swdge_queues=4)

    xlo_d = nc.declare_dram_parameter("xlo", [NLO, F], bf16, isOutput=False)
    xhi_d = nc.declare_dram_parameter("xhi", [NHI, F], bf16, isOutput=False)
    idx_d = nc.declare_dram_parameter("idx", [128, BPC * ICOLS], i16, isOutput=False)
    dloc_d = nc.declare_dram_parameter("dloc", [128, NCHUNK, 1], f32, isOutput=False)
    vloc_d = nc.declare_dram_parameter("vloc", [128, NCHUNK, 1], f32, isOutput=False)
    wt_d = nc.declare_dram_parameter("wt", [128, 4, F], bf16, isOutput=False)
    iota_d = nc.declare_dram_parameter("iota", [128, T, 128], bf16, isOutput=False)
    ident_d = nc.declare_dram_parameter("ident", [128, 128], bf16, isOutput=False)
    out_d = nc.declare_dram_parameter("out", [BPC * 128, F], f32, isOutput=True)

    with tile.TileContext(nc) as tc:
        with (
            tc.tile_pool(name="const", bufs=1) as cp,
            tc.tile_pool(name="msgp", bufs=4) as mp,
            tc.tile_pool(name="indp", bufs=3) as ip,
            tc.tile_pool(name="smallp", bufs=3) as sp,
            tc.tile_pool(name="accp", bufs=2, space="PSUM") as pa,
            tc.tile_pool(name="tpp", bufs=2, space="PSUM") as pt,
            tc.tile_pool(name="o2p", bufs=2, space="PSUM") as po,
        ):
            idx_t = cp.tile([128, BPC * ICOLS], i16)
            nc.sync.dma_start(idx_t[:], idx_d[:])
            dloc_t = cp.tile([128, NCHUNK, 1], f32)
            nc.sync.dma_start(dloc_t[:], dloc_d[:])
            vloc_t = cp.tile([128, NCHUNK, 1], f32)
            nc.sync.dma_start(vloc_t[:], vloc_d[:])
            wt_t = cp.tile([128, 4, F], bf16)
            nc.sync.dma_start(wt_t[:], wt_d[:])
            iota_t = cp.tile([128, T, 128], bf16)
            nc.sync.dma_start(iota_t[:], iota_d[:])
            ident_t = cp.tile([128, 128], bf16)
            nc.sync.dma_start(ident_t[:], ident_d[:])

            for b in range(BPC):
                msg = mp.tile([128, T, F], bf16)
                ib = b * ICOLS
                # the gather ucode handles at most 1024 indices (64 descs)
                # per instruction -> split the lo gather 1024 + 384
                nc.gpsimd.dma_gather(
                    msg[:, 0:8, :], xlo_d[:],
                    idx_t[:, ib : ib + 64], 1024, 1024, F,
                    queue_num=(b * 3) % 4,
                )
                nc.gpsimd.dma_gather(
                    msg[:, 8:T_LO, :], xlo_d[:],
                    idx_t[:, ib + 64 : ib + ICOL_LO], CAP_LO - 1024, CAP_LO - 1024, F,
                    queue_num=(b * 3 + 1) % 4,
                )
                nc.gpsimd.dma_gather(
                    msg[:, T_LO:T, :], xhi_d[:],
                    idx_t[:, ib + ICOL_LO : ib + ICOLS], CAP_HI, CAP_HI, F,
                    queue_num=(b * 3 + 2) % 4,
                )
                ind = ip.tile([128, T, 128], bf16)
                cb = b * T
                nc.vector.tensor_tensor(
                    ind[:], dloc_t[:, cb : cb + T, :].to_broadcast([128, T, 128]),
                    iota_t[:], OP.is_equal,
                )
                nc.vector.tensor_tensor(
                    ind[:], ind[:],
                    vloc_t[:, cb : cb + T, :].to_broadcast([128, T, 128]),
                    OP.mult,
                )
                acc = pa.tile([128, F], f32)
                for t in range(T):
                    nc.tensor.matmul(
                        acc[:], ind[:, t, :], msg[:, t, :],
                        start=(t == 0), stop=(t == T - 1),
                    )
                ag = sp.tile([128, F], bf16, tag="ag")
                nc.vector.tensor_copy(ag[:], acc[:])
                tp = pt.tile([128, F], bf16)
                for k in range(4):
                    nc.tensor.transpose(
                        tp[:, k * 128 : (k + 1) * 128],
                        ag[:, k * 128 : (k + 1) * 128],
                        ident_t[:],
                    )
                tpsb = sp.tile([128, F], bf16, tag="tpsb")
                nc.vector.tensor_copy(tpsb[:], tp[:])
                o2 = po.tile([128, F], f32)
                for k in range(4):
                    nc.tensor.matmul(
                        o2[:], tpsb[:, k * 128 : (k + 1) * 128], wt_t[:, k, :],
                        start=(k == 0), stop=(k == 3),
                    )
                osb = sp.tile([128, F], f32, tag="osb")
                nc.scalar.activation(osb[:], o2[:], AF.Relu)
                nc.sync.dma_start(out_d[b * 128 : (b + 1) * 128, :], osb[:])
    nc.compile()
    return nc


def _get_nc():
    if "nc" not in _compiled:
        _compiled["nc"] = _build_nc()
    return _compiled["nc"]


def _prep_inputs(X, W, edge_src, edge_dst, edge_vals):
    """Host layout: permutation, edge sort/pad, index packing. Returns
    (in_maps, pos) where pos[node] is its row in the concatenated output."""
    import ml_dtypes

    bf = ml_dtypes.bfloat16

    src = np.asarray(edge_src).astype(np.int64)
    dst = np.asarray(edge_dst).astype(np.int64)
    vals = np.asarray(edge_vals).astype(np.float32)
    X = np.asarray(X, dtype=np.float32)
    W = np.asarray(W, dtype=np.float32)

    islo = src < SPLIT
    deg_lo = np.bincount(dst[islo], minlength=N)
    deg_lo_full = np.concatenate([deg_lo, np.zeros(PADN - N, np.int64)])
    order = np.argsort(-deg_lo_full, kind="stable")
    r = np.arange(PADN)
    row = r // BLOCKS
    col = r % BLOCKS
    blk = np.where(row % 2 == 0, col, BLOCKS - 1 - col)
    block_of = np.empty(PADN, np.int64)
    slot_of = np.empty(PADN, np.int64)
    block_of[order] = blk
    slot_of[order] = row
    pos = block_of * 128 + slot_of

    b_e = block_of[dst]
    g_e = b_e * 2 + (~islo).astype(np.int64)
    eo = np.argsort(g_e, kind="stable")
    g_s = g_e[eo]
    src_s = src[eo]
    dst_s = dst[eo]
    val_s = vals[eo]
    counts = np.bincount(g_e, minlength=BLOCKS * 2)
    n_lo_max = counts[0::2].max()
    n_hi_max = counts[1::2].max()
    assert n_lo_max <= CAP_LO and n_hi_max <= CAP_HI, (
        f"block group overflow: lo {n_lo_max}/{CAP_LO} hi {n_hi_max}/{CAP_HI}"
    )
    starts = np.zeros(BLOCKS * 2, np.int64)
    starts[1:] = np.cumsum(counts)[:-1]
    rank = np.arange(E) - starts[g_s]
    b_s = g_s // 2
    hi_s = g_s % 2
    core_s = b_s // BPC
    bb_s = b_s % BPC

    mslot = hi_s * CAP_LO + rank
    chunk = bb_s * T + mslot // 128
    part = mslot % 128
    icol = bb_s * ICOLS + hi_s * ICOL_LO + rank // 16
    ipart = rank % 16
    idxval = np.where(hi_s == 1, src_s - SPLIT, src_s).astype(np.int16)

    idx_base = np.zeros((CORES, 16, BPC * ICOLS), np.int16)
    idx_base[core_s, ipart, icol] = idxval
    idx = np.ascontiguousarray(np.tile(idx_base, (1, 8, 1)))
    dloc = np.zeros((CORES, 128, NCHUNK), dtype=np.float32)
    vloc = np.zeros((CORES, 128, NCHUNK), dtype=np.float32)
    dloc[core_s, part, chunk] = slot_of[dst_s].astype(np.float32)
    vloc[core_s, part, chunk] = val_s.astype(np.float32)
    dloc = dloc.reshape(CORES, 128, NCHUNK, 1)
    vloc = vloc.reshape(CORES, 128, NCHUNK, 1)

    xlo = np.ascontiguousarray(X[:SPLIT].astype(bf))
    xhi = np.ascontiguousarray(X[SPLIT:].astype(bf))
    wt = np.ascontiguousarray(
        W.astype(bf).reshape(4, 128, F).transpose(1, 0, 2)
    )
    iota = np.ascontiguousarray(
        np.broadcast_to(np.tile(np.arange(128), T), (128, T * 128))
        .reshape(128, T, 128).astype(bf)
    )
    ident = np.eye(128, dtype=bf)

    in_maps = [
        {
            "xlo": xlo,
            "xhi": xhi,
            "idx": np.ascontiguousarray(idx[c]),
            "dloc": np.ascontiguousarray(dloc[c]),
            "vloc": np.ascontiguousarray(vloc[c]),
            "wt": wt,
            "iota": iota,
            "ident": ident,
        }
        for c in range(CORES)
    ]
    return in_maps, pos


def kernel(X, W, edge_src, edge_dst, edge_vals):
    from concourse.bass_utils import run_bass_kernel_spmd

    in_maps, pos = _prep_inputs(X, W, edge_src, edge_dst, edge_vals)
    nc = _get_nc()
    res = run_bass_kernel_spmd(nc, in_maps, core_ids=list(range(CORES)))
    outs = res.results
    full = np.concatenate(
        [np.asarray(outs[c]["out"]) for c in range(CORES)], axis=0
    )  # [PADN, F]
    return np.ascontiguousarray(full[pos[:N]]).astype(np.float32)


# revision 7
# speedup vs baseline: 218.9945x; 1.2299x over previous
"""GCN layer (A_hat @ (X W) with ReLU) fully on-device for Trainium2, 8 cores.

Math: out = relu(segment_sum(Z[edge_src] * edge_vals, edge_dst)) with Z = X@W.
We use the equivalent order out = relu((A @ X) @ W) so the gather table is the
input X (replicated to every core) and no cross-core collective is needed.

Sharding: destination nodes are sharded across the 8 cores (graph
partitioning per the hint). Each core owns 49 blocks of 128 dst nodes.
Per block, the device:
  1. bulk-gathers the source rows of the block's edges from X (bf16) via
     two dma_gather calls (X is split in two tables so row indices fit the
     gather engine's int16 index format),
  2. builds a [128 edges x 128 dst] indicator matrix per 128-edge chunk on
     the VectorEngine (indicator[e,d] = edge_val[e] if dst_local[e]==d),
  3. accumulates agg = indicator^T @ msg on the TensorEngine into PSUM
     (the segment-sum becomes a chain of matmuls, race-free),
  4. transposes agg (TensorE) and multiplies by W, applies ReLU (ScalarE),
     writes the finished [128, 512] output rows once. No scatter anywhere.

Host work is only layout: degree-balanced dst->block permutation, edge
sorting/padding, int16 index packing, and the inverse row permutation on
the way out.
"""

import numpy as np

N = 50000
F = 512
E = 800000
CORES = 8
SPLIT = 32768          # X row split so gather indices fit int16
NLO = SPLIT
NHI = N - SPLIT        # 17232
BPC = 49               # dst blocks per core
BLOCKS = CORES * BPC   # 392
PADN = BLOCKS * 128    # 50176
T_LO, T_HI = 11, 7     # static 128-edge chunks per block per table
T = T_LO + T_HI
CAP_LO, CAP_HI = T_LO * 128, T_HI * 128
SLOTS = T * 128
ICOL_LO, ICOL_HI = CAP_LO // 16, CAP_HI // 16
ICOLS = ICOL_LO + ICOL_HI
NCHUNK = BPC * T

_compiled = {}


def _build_nc():
    from concourse import bacc, mybir
    from concourse import tile

    f32 = mybir.dt.float32
    bf16 = mybir.dt.bfloat16
    i16 = mybir.dt.int16
    AF = mybir.ActivationFunctionType
    OP = mybir.AluOpType

    nc = bacc.Bacc(None, debug=False, num_swdge_queues=4)

    xlo_d = nc.declare_dram_parameter("xlo", [NLO, F], bf16, isOutput=False)
    xhi_d = nc.declare_dram_parameter("xhi", [NHI, F], bf16, isOutput=False)
    idx_d = nc.declare_dram_parameter("idx", [128, BPC * ICOLS], i16, isOutput=False)
    dloc_d = nc.declare_dram_parameter("dloc", [128, NCHUNK, 1], bf16, isOutput=False)
    vloc_d = nc.declare_dram_parameter("vloc", [128, NCHUNK, 1], bf16, isOutput=False)
    wt_d = nc.declare_dram_parameter("wt", [128, 4, F], bf16, isOutput=False)
    iota_d = nc.declare_dram_parameter("iota", [128, T, 128], bf16, isOutput=False)
    ident_d = nc.declare_dram_parameter("ident", [128, 128], bf16, isOutput=False)
    out_d = nc.declare_dram_parameter("out", [BPC * 128, F], bf16, isOutput=True)

    with tile.TileContext(nc) as tc:
        with (
            tc.tile_pool(name="const", bufs=1) as cp,
            tc.tile_pool(name="msgp", bufs=6) as mp,
            tc.tile_pool(name="indp", bufs=4) as ip,
            tc.tile_pool(name="smallp", bufs=3) as sp,
            tc.tile_pool(name="accp", bufs=2, space="PSUM") as pa,
            tc.tile_pool(name="tpp", bufs=2, space="PSUM") as pt,
            tc.tile_pool(name="o2p", bufs=2, space="PSUM") as po,
        ):
            idx_t = cp.tile([128, BPC * ICOLS], i16)
            nc.sync.dma_start(idx_t[:], idx_d[:])
            dloc_t = cp.tile([128, NCHUNK, 1], bf16)
            nc.sync.dma_start(dloc_t[:], dloc_d[:])
            vloc_t = cp.tile([128, NCHUNK, 1], bf16)
            nc.sync.dma_start(vloc_t[:], vloc_d[:])
            wt_t = cp.tile([128, 4, F], bf16)
            nc.sync.dma_start(wt_t[:], wt_d[:])
            iota_t = cp.tile([128, T, 128], bf16)
            nc.sync.dma_start(iota_t[:], iota_d[:])
            ident_t = cp.tile([128, 128], bf16)
            nc.sync.dma_start(ident_t[:], ident_d[:])

            for b in range(BPC):
                msg = mp.tile([128, T, F], bf16)
                ib = b * ICOLS
                # the gather ucode handles at most 1024 indices (64 descs)
                # per instruction -> split the lo gather 1024 + 384
                nc.gpsimd.dma_gather(
                    msg[:, 0:8, :], xlo_d[:],
                    idx_t[:, ib : ib + 64], 1024, 1024, F,
                    queue_num=(b * 3) % 4,
                )
                nc.gpsimd.dma_gather(
                    msg[:, 8:T_LO, :], xlo_d[:],
                    idx_t[:, ib + 64 : ib + ICOL_LO], CAP_LO - 1024, CAP_LO - 1024, F,
                    queue_num=(b * 3 + 1) % 4,
                )
                nc.gpsimd.dma_gather(
                    msg[:, T_LO:T, :], xhi_d[:],
                    idx_t[:, ib + ICOL_LO : ib + ICOLS], CAP_HI, CAP_HI, F,
                    queue_num=(b * 3 + 2) % 4,
                )
                ind = ip.tile([128, T, 128], bf16)
                cb = b * T
                nc.vector.tensor_tensor(
                    ind[:], dloc_t[:, cb : cb + T, :].to_broadcast([128, T, 128]),
                    iota_t[:], OP.is_equal,
                )
                nc.vector.tensor_tensor(
                    ind[:], ind[:],
                    vloc_t[:, cb : cb + T, :].to_broadcast([128, T, 128]),
                    OP.mult,
                )
                acc = pa.tile([128, F], f32)
                for t in range(T):
                    nc.tensor.matmul(
                        acc[:], ind[:, t, :], msg[:, t, :],
                        start=(t == 0), stop=(t == T - 1),
                    )
                ag = sp.tile([128, F], bf16, tag="ag")
                nc.vector.tensor_copy(ag[:], acc[:])
                tp = pt.tile([128, F], bf16)
                for k in range(4):
                    nc.tensor.transpose(
                        tp[:, k * 128 : (k + 1) * 128],
                        ag[:, k * 128 : (k + 1) * 128],
                        ident_t[:],
                    )
                tpsb = sp.tile([128, F], bf16, tag="tpsb")
                nc.vector.tensor_copy(tpsb[:], tp[:])
                o2 = po.tile([128, F], f32)
                for k in range(4):
                    nc.tensor.matmul(
                        o2[:], tpsb[:, k * 128 : (k + 1) * 128], wt_t[:, k, :],
                        start=(k == 0), stop=(k == 3),
                    )
                osb = sp.tile([128, F], bf16, tag="osb")
                nc.scalar.activation(osb[:], o2[:], AF.Relu)
                nc.sync.dma_start(out_d[b * 128 : (b + 1) * 128, :], osb[:])
    nc.compile()
    return nc


def _get_nc():
    if "nc" not in _compiled:
        _compiled["nc"] = _build_nc()
    return _compiled["nc"]


def _prep_inputs(X, W, edge_src, edge_dst, edge_vals):
    """Host layout: permutation, edge sort/pad, index packing. Returns
    (in_maps, pos) where pos[node] is its row in the concatenated output."""
    import ml_dtypes

    bf = ml_dtypes.bfloat16

    src = np.asarray(edge_src).astype(np.int64)
    dst = np.asarray(edge_dst).astype(np.int64)
    vals = np.asarray(edge_vals).astype(np.float32)
    X = np.asarray(X, dtype=np.float32)
    W = np.asarray(W, dtype=np.float32)

    islo = src < SPLIT
    deg_lo = np.bincount(dst[islo], minlength=N)
    deg_lo_full = np.concatenate([deg_lo, np.zeros(PADN - N, np.int64)])
    order = np.argsort(-deg_lo_full, kind="stable")
    r = np.arange(PADN)
    row = r // BLOCKS
    col = r % BLOCKS
    blk = np.where(row % 2 == 0, col, BLOCKS - 1 - col)
    block_of = np.empty(PADN, np.int64)
    slot_of = np.empty(PADN, np.int64)
    block_of[order] = blk
    slot_of[order] = row
    pos = block_of * 128 + slot_of

    b_e = block_of[dst]
    g_e = b_e * 2 + (~islo).astype(np.int64)
    eo = np.argsort(g_e, kind="stable")
    g_s = g_e[eo]
    src_s = src[eo]
    dst_s = dst[eo]
    val_s = vals[eo]
    counts = np.bincount(g_e, minlength=BLOCKS * 2)
    n_lo_max = counts[0::2].max()
    n_hi_max = counts[1::2].max()
    assert n_lo_max <= CAP_LO and n_hi_max <= CAP_HI, (
        f"block group overflow: lo {n_lo_max}/{CAP_LO} hi {n_hi_max}/{CAP_HI}"
    )
    starts = np.zeros(BLOCKS * 2, np.int64)
    starts[1:] = np.cumsum(counts)[:-1]
    rank = np.arange(E) - starts[g_s]
    b_s = g_s // 2
    hi_s = g_s % 2
    core_s = b_s // BPC
    bb_s = b_s % BPC

    mslot = hi_s * CAP_LO + rank
    chunk = bb_s * T + mslot // 128
    part = mslot % 128
    icol = bb_s * ICOLS + hi_s * ICOL_LO + rank // 16
    ipart = rank % 16
    idxval = np.where(hi_s == 1, src_s - SPLIT, src_s).astype(np.int16)

    idx_base = np.zeros((CORES, 16, BPC * ICOLS), np.int16)
    idx_base[core_s, ipart, icol] = idxval
    idx = np.ascontiguousarray(np.tile(idx_base, (1, 8, 1)))
    dloc = np.zeros((CORES, 128, NCHUNK), dtype=np.float32)
    vloc = np.zeros((CORES, 128, NCHUNK), dtype=np.float32)
    dloc[core_s, part, chunk] = slot_of[dst_s].astype(np.float32)
    vloc[core_s, part, chunk] = val_s.astype(np.float32)
    dloc = dloc.reshape(CORES, 128, NCHUNK, 1).astype(bf)
    vloc = vloc.reshape(CORES, 128, NCHUNK, 1).astype(bf)

    xlo = np.ascontiguousarray(X[:SPLIT].astype(bf))
    xhi = np.ascontiguousarray(X[SPLIT:].astype(bf))
    wt = np.ascontiguousarray(
        W.astype(bf).reshape(4, 128, F).transpose(1, 0, 2)
    )
    iota = np.ascontiguousarray(
        np.broadcast_to(np.tile(np.arange(128), T), (128, T * 128))
        .reshape(128, T, 128).astype(bf)
    )
    ident = np.eye(128, dtype=bf)

    in_maps = [
        {
            "xlo": xlo,
            "xhi": xhi,
            "idx": np.ascontiguousarray(idx[c]),
            "dloc": np.ascontiguousarray(dloc[c]),
            "vloc": np.ascontiguousarray(vloc[c]),
            "wt": wt,
            "iota": iota,
            "ident": ident,
        }
        for c in range(CORES)
    ]
    return in_maps, pos


def kernel(X, W, edge_src, edge_dst, edge_vals):
    from concourse.bass_utils import run_bass_kernel_spmd

    in_maps, pos = _prep_inputs(X, W, edge_src, edge_dst, edge_vals)
    nc = _get_nc()
    res = run_bass_kernel_spmd(nc, in_maps, core_ids=list(range(CORES)))
    outs = res.results
    full = np.concatenate(
        [np.asarray(outs[c]["out"]) for c in range(CORES)], axis=0
    )  # [PADN, F]
    return np.ascontiguousarray(full[pos[:N]]).astype(np.float32)
